# revision 3
# baseline (speedup 1.0000x reference)
"""Depthwise 1d (per-channel linear) Trainium2 Bass kernel.

out[n, c, o] = sum_i x[n, c, i] * W[c, o, i] + b[c, o]
  x: [4096, 256, 64] f32, W: [256, 128, 64] f32, b: [256, 128] f32
  out: [4096, 256, 128] f32

Strategy: shard channels across 8 cores (32 channels/core, all 4096 rows).
Per-core weights are a single 1 MB block-diagonal fp16 tile loaded once,
so steady state moves only x in (33.5 MB) and out (67 MB) -- the kernel
is HBM-bound at ~358 GB/s/core.

Per n-tile of 128 rows: x loads naturally as [n, (c,i)]; PE-transposes
of 128x128 chunks (2 channels each) give lhsT = [(2ch x 64i), n]; each
channel pair is one fp16 matmul against a block-diagonal rhs [128, 256]
(upper-left = W_c0.T, lower-right = W_c1.T), keeping the full 128-row
contraction busy.  fp16 is ample precision for the 2e-2 gate (~1e-4).
The fp32->fp16 cast is fused into the PSUM evacuation of the transposes
on the ScalarE; bias is added on the DVE during PSUM evacuation of the
matmul results, against a partition-broadcast bias tile built once at
startup.  The PE stream is software-pipelined one iteration deep so
transposes for tile k+1 interleave ahead of tile k's matmuls.  Output
stores ride the ACT HWDGE ring so x loads never queue behind them.
"""

import os

# recover cleanly if a previous run left the NeuronCores wedged; must be
# set before the runtime initializes
os.environ.setdefault("NEURON_RT_RESET_CORES", "1")

import numpy as np

import concourse.bass as bass
import concourse.tile as tile
from concourse import bacc, mybir
from concourse.bass_utils import run_bass_kernel_spmd

N_CORES = 8
N, C, HI, HO = 4096, 256, 64, 128
CLOC = C // N_CORES  # 32 channels per core
NT = 128             # batch rows per tile

F32 = mybir.dt.float32
F16 = mybir.dt.float16


def build(n=N, cloc=CLOC, n_cores=N_CORES):
    nc = bacc.Bacc(
        "TRN2", target_bir_lowering=False, debug=False, num_devices=n_cores
    )
    pairs = cloc // 2
    x_d = nc.dram_tensor("x", [n, cloc, HI], F32, kind="ExternalInput").ap()
    # block-diagonal fp16 weights, host-packed: row k=(h*64+i), col block h
    w_d = nc.dram_tensor("wbd", [128, pairs, 2 * HO], F16, kind="ExternalInput").ap()
    b_d = nc.dram_tensor("bias", [cloc, HO], F32, kind="ExternalInput").ap()
    i_d = nc.dram_tensor("ident", [128, 128], F32, kind="ExternalInput").ap()
    o_d = nc.dram_tensor("out", [n, cloc, HO], F32, kind="ExternalOutput").ap()

    n_tiles = n // NT

    with tile.TileContext(nc) as tc:
        with (
            tc.tile_pool(name="const", bufs=1) as const,
            tc.tile_pool(name="xp", bufs=4) as xp,
            tc.tile_pool(name="xhp", bufs=4) as xhp,
            tc.tile_pool(name="op", bufs=4) as op,
            tc.tile_pool(name="pst", bufs=3, space="PSUM") as pst,
            tc.tile_pool(name="pso", bufs=5, space="PSUM") as pso,
        ):
            ident = const.tile([128, 128], F32, tag="ident")
            nc.sync.dma_start(out=ident, in_=i_d)
            wt = const.tile([128, pairs, 2 * HO], F16, tag="wt")
            nc.gpsimd.dma_start(out=wt, in_=w_d)
            b_one = const.tile([1, cloc, HO], F32, tag="b_one")
            nc.gpsimd.dma_start(out=b_one, in_=b_d)
            bias_sb = const.tile([128, cloc, HO], F32, tag="bias_sb")
            nc.gpsimd.partition_broadcast(bias_sb, b_one)

            def emit_T(ni):
                # x load, fp32 transposes, fused fp16 cast on PSUM
                # evacuation (ACT)
                n0 = ni * NT
                x_sb = xp.tile([128, cloc, HI], F32, name=f"x{ni}", tag="x")
                nc.sync.dma_start(out=x_sb, in_=x_d[n0 : n0 + NT, :, :])
                xh_sb = xhp.tile([128, pairs, NT], F16, name=f"xh{ni}", tag="xh")
                for g in range(pairs // 4):  # 4 fp32 pairs per PSUM bank
                    ps = pst.tile([128, 4, NT], F32)
                    for q in range(4):
                        j = g * 4 + q
                        nc.tensor.transpose(
                            ps[:, q, :], x_sb[:, 2 * j : 2 * j + 2, :], ident
                        )
                    sl = slice(g * 4, (g + 1) * 4)
                    nc.scalar.copy(out=xh_sb[:, sl, :], in_=ps)
                return xh_sb

            def emit_M(ni, xh_sb):
                n0 = ni * NT
                o_sb = op.tile([128, cloc, HO], F32)
                for g in range(cloc // 4):  # 4 channels / 2 pairs per bank
                    po = pso.tile([128, 4, HO], F32)
                    for p in range(2):
                        j = g * 2 + p
                        nc.tensor.matmul(
                            po[:, 2 * p : 2 * p + 2, :],
                            lhsT=xh_sb[:, j, :],
                            rhs=wt[:, j, :],
                            start=True,
                            stop=True,
                        )
                    nc.vector.tensor_add(
                        out=o_sb[:, g * 4 : (g + 1) * 4, :],
                        in0=po,
                        in1=bias_sb[:, g * 4 : (g + 1) * 4, :],
                    )
                nc.scalar.dma_start(out=o_d[n0 : n0 + NT, :, :], in_=o_sb)

            staged = emit_T(0)
            for ni in range(n_tiles):
                cur = staged
                # pipeline: next tile's transposes go to the PE ahead of
                # this tile's matmuls
                if ni + 1 < n_tiles:
                    staged = emit_T(ni + 1)
                emit_M(ni, cur)
    nc.compile()
    return nc


def pack_w(W):
    """[C, HO, HI] -> per-core block-diagonal [8, 128, C//16, 256] fp16."""
    C_, HO_, HI_ = W.shape
    pairs = C_ // (2 * N_CORES)
    Wt = W.astype(np.float16).transpose(0, 2, 1)  # [C, HI, HO] = W_c.T
    Wr = Wt.reshape(N_CORES, pairs, 2, HI_, HO_)
    out = np.zeros((N_CORES, 2, HI_, pairs, 2, HO_), dtype=np.float16)
    out[:, 0, :, :, 0, :] = Wr[:, :, 0].transpose(0, 2, 1, 3)
    out[:, 1, :, :, 1, :] = Wr[:, :, 1].transpose(0, 2, 1, 3)
    return np.ascontiguousarray(out.reshape(N_CORES, 128, pairs, 2 * HO_))


_cache = {}


def kernel(x, W, b):
    nc = _cache.get("nc")
    if nc is None:
        nc = _cache["nc"] = build()
    xs = np.asarray(x, dtype=np.float32)
    Wbd = pack_w(np.asarray(W, dtype=np.float32))
    bb = np.asarray(b, dtype=np.float32)
    ident = np.eye(128, dtype=np.float32)
    in_maps = [
        {
            "x": np.ascontiguousarray(xs[:, i * CLOC : (i + 1) * CLOC]),
            "wbd": Wbd[i],
            "bias": np.ascontiguousarray(bb[i * CLOC : (i + 1) * CLOC]),
            "ident": ident,
        }
        for i in range(N_CORES)
    ]
    res = run_bass_kernel_spmd(nc, in_maps, core_ids=list(range(N_CORES)))
    return np.concatenate(
        [res.results[i]["out"] for i in range(N_CORES)], axis=1
    )


# revision 5
# speedup vs baseline: 1.1153x; 1.1153x over previous
"""Depthwise 1d (per-channel linear) Trainium2 Bass kernel.

out[n, c, o] = sum_i x[n, c, i] * W[c, o, i] + b[c, o]
  x: [4096, 256, 64] f32, W: [256, 128, 64] f32, b: [256, 128] f32
  out: [4096, 256, 128] f32

Strategy: shard channels across 8 cores (32 channels/core, all 4096 rows).
Per-core weights are a single 1 MB block-diagonal fp16 tile loaded once,
so steady state moves only x in (33.5 MB) and out (67 MB) -- the kernel
is HBM-bound at ~358 GB/s/core.

Per n-tile of 128 rows: x loads naturally as [n, (c,i)]; PE-transposes
of 128x128 chunks (2 channels each) give lhsT = [(2ch x 64i), n]; each
channel pair is one fp16 matmul against a block-diagonal rhs [128, 256]
(upper-left = W_c0.T, lower-right = W_c1.T), keeping the full 128-row
contraction busy.  fp16 is ample precision for the 2e-2 gate (~1e-4).
The fp32->fp16 cast is fused into the PSUM evacuation of the transposes
on the ScalarE; bias is added on the DVE during PSUM evacuation of the
matmul results, against a partition-broadcast bias tile built once at
startup.  The PE stream is software-pipelined one iteration deep so
transposes for tile k+1 interleave ahead of tile k's matmuls.  Output
stores ride the ACT HWDGE ring so x loads never queue behind them.
"""

import os

# recover cleanly if a previous run left the NeuronCores wedged; must be
# set before the runtime initializes
os.environ.setdefault("NEURON_RT_RESET_CORES", "1")

import numpy as np

import concourse.bass as bass
import concourse.tile as tile
from concourse import bacc, mybir
from concourse.bass_utils import run_bass_kernel_spmd

N_CORES = 8
N, C, HI, HO = 4096, 256, 64, 128
CLOC = C // N_CORES  # 32 channels per core
NT = 128             # batch rows per tile

F32 = mybir.dt.float32
F16 = mybir.dt.float16


def build(n=N, cloc=CLOC, n_cores=N_CORES):
    nc = bacc.Bacc(
        "TRN2", target_bir_lowering=False, debug=False, num_devices=n_cores
    )
    pairs = cloc // 2
    x_d = nc.dram_tensor("x", [n, cloc, HI], F32, kind="ExternalInput").ap()
    # block-diagonal fp16 weights, host-packed: row k=(h*64+i), col block h
    w_d = nc.dram_tensor("wbd", [128, pairs, 2 * HO], F16, kind="ExternalInput").ap()
    b_d = nc.dram_tensor("bias", [cloc, HO], F32, kind="ExternalInput").ap()
    i_d = nc.dram_tensor("ident", [128, 128], F32, kind="ExternalInput").ap()
    o_d = nc.dram_tensor("out", [n, cloc, HO], F32, kind="ExternalOutput").ap()

    n_tiles = n // NT

    with tile.TileContext(nc) as tc:
        with (
            tc.tile_pool(name="const", bufs=1) as const,
            tc.tile_pool(name="xp", bufs=4) as xp,
            tc.tile_pool(name="xhp", bufs=4) as xhp,
            tc.tile_pool(name="op", bufs=4) as op,
            tc.tile_pool(name="pst", bufs=3, space="PSUM") as pst,
            tc.tile_pool(name="pso", bufs=5, space="PSUM") as pso,
        ):
            # first x tile load goes out before the constants so the DMA
            # engines ramp on the bulk stream immediately
            x_first = xp.tile([128, cloc, HI], F32, name="x0", tag="x")
            nc.sync.dma_start(out=x_first, in_=x_d[0:NT, :, :])

            ident = const.tile([128, 128], F32, tag="ident")
            nc.sync.dma_start(out=ident, in_=i_d)
            wt = const.tile([128, pairs, 2 * HO], F16, tag="wt")
            nc.gpsimd.dma_start(out=wt, in_=w_d)
            b_one = const.tile([1, cloc, HO], F32, tag="b_one")
            nc.gpsimd.dma_start(out=b_one, in_=b_d)
            bias_sb = const.tile([128, cloc, HO], F32, tag="bias_sb")
            nc.gpsimd.partition_broadcast(bias_sb, b_one)

            def emit_T(ni):
                # x load, fp32 transposes, fused fp16 cast on PSUM
                # evacuation (ACT)
                n0 = ni * NT
                if ni == 0:
                    x_sb = x_first
                else:
                    x_sb = xp.tile([128, cloc, HI], F32, name=f"x{ni}", tag="x")
                    nc.sync.dma_start(out=x_sb, in_=x_d[n0 : n0 + NT, :, :])
                xh_sb = xhp.tile([128, pairs, NT], F16, name=f"xh{ni}", tag="xh")
                for g in range(pairs // 4):  # 4 fp32 pairs per PSUM bank
                    ps = pst.tile([128, 4, NT], F32)
                    for q in range(4):
                        j = g * 4 + q
                        nc.tensor.transpose(
                            ps[:, q, :], x_sb[:, 2 * j : 2 * j + 2, :], ident
                        )
                    sl = slice(g * 4, (g + 1) * 4)
                    nc.scalar.copy(out=xh_sb[:, sl, :], in_=ps)
                return xh_sb

            def emit_M(ni, xh_sb):
                n0 = ni * NT
                o_sb = op.tile([128, cloc, HO], F32)
                half = cloc // 8  # matmul groups per half-tile store
                for g in range(cloc // 4):  # 4 channels / 2 pairs per bank
                    po = pso.tile([128, 4, HO], F32)
                    for p in range(2):
                        j = g * 2 + p
                        nc.tensor.matmul(
                            po[:, 2 * p : 2 * p + 2, :],
                            lhsT=xh_sb[:, j, :],
                            rhs=wt[:, j, :],
                            start=True,
                            stop=True,
                        )
                    nc.vector.tensor_add(
                        out=o_sb[:, g * 4 : (g + 1) * 4, :],
                        in0=po,
                        in1=bias_sb[:, g * 4 : (g + 1) * 4, :],
                    )
                    if (g + 1) % half == 0:
                        # store each half as soon as its adds land, so the
                        # store stream feeds the DMA engines smoothly
                        c0 = (g + 1 - half) * 4
                        c1 = (g + 1) * 4
                        nc.scalar.dma_start(
                            out=o_d[n0 : n0 + NT, c0:c1, :],
                            in_=o_sb[:, c0:c1, :],
                        )

            staged = emit_T(0)
            for ni in range(n_tiles):
                cur = staged
                # pipeline: next tile's transposes go to the PE ahead of
                # this tile's matmuls
                if ni + 1 < n_tiles:
                    staged = emit_T(ni + 1)
                emit_M(ni, cur)
    nc.compile()
    return nc


def pack_w(W):
    """[C, HO, HI] -> per-core block-diagonal [8, 128, C//16, 256] fp16."""
    C_, HO_, HI_ = W.shape
    pairs = C_ // (2 * N_CORES)
    Wt = W.astype(np.float16).transpose(0, 2, 1)  # [C, HI, HO] = W_c.T
    Wr = Wt.reshape(N_CORES, pairs, 2, HI_, HO_)
    out = np.zeros((N_CORES, 2, HI_, pairs, 2, HO_), dtype=np.float16)
    out[:, 0, :, :, 0, :] = Wr[:, :, 0].transpose(0, 2, 1, 3)
    out[:, 1, :, :, 1, :] = Wr[:, :, 1].transpose(0, 2, 1, 3)
    return np.ascontiguousarray(out.reshape(N_CORES, 128, pairs, 2 * HO_))


_cache = {}


def kernel(x, W, b):
    nc = _cache.get("nc")
    if nc is None:
        nc = _cache["nc"] = build()
    xs = np.asarray(x, dtype=np.float32)
    Wbd = pack_w(np.asarray(W, dtype=np.float32))
    bb = np.asarray(b, dtype=np.float32)
    ident = np.eye(128, dtype=np.float32)
    in_maps = [
        {
            "x": np.ascontiguousarray(xs[:, i * CLOC : (i + 1) * CLOC]),
            "wbd": Wbd[i],
            "bias": np.ascontiguousarray(bb[i * CLOC : (i + 1) * CLOC]),
            "ident": ident,
        }
        for i in range(N_CORES)
    ]
    res = run_bass_kernel_spmd(nc, in_maps, core_ids=list(range(N_CORES)))
    return np.concatenate(
        [res.results[i]["out"] for i in range(N_CORES)], axis=1
    )
